# revision 1
# baseline (speedup 1.0000x reference)
"""Causal self-attention (B=2,T=2048,D=1024,H=16,HD=64) + RoPE on 8 TRN2 NeuronCores.

Sharding: core = b*4 + g  (b: batch, g: head-group of 4 heads).
Each core computes QKV projection for its 4 heads, causal attention, and a
partial out-projection (rank-256 contribution). Host sums the 4 partials per
batch (the "all-reduce after out_proj").

All matmuls run as float32r (TF32-like, ~1e-4 rel err) on the PE array.
Program order interleaves QKV chunks with attention groups so the PE stream
stays dense (HAM stays un-throttled) and ACT exp overlaps PE matmuls.
"""
import numpy as np

import concourse.bass as bass
import concourse.mybir as mybir
from concourse import bacc
from concourse.tile import TileContext
from concourse.bass_utils import run_bass_kernel_spmd

B, T, D, H = 2, 2048, 1024, 16
HD = D // H            # 64
G = 4                  # head groups (tensor-parallel factor)
HPG = H // G           # 4 heads per group
DG = HPG * HD          # 256 head-dims per group
KC = D // 128          # 8 contraction chunks for D
NT = T // 512          # 4 T-chunks of 512
TT = T // 128          # 16 T-tiles of 128
F32 = mybir.dt.float32
F32R = mybir.dt.float32r
SWAP16 = [(i + 16) % 32 for i in range(32)]  # e<->o halves within each 32-quadrant

_CACHE = {}
DEBUG_DUMPS = False


def _build():
    nc = bacc.Bacc("TRN2", target_bir_lowering=False, debug=False, num_devices=8)

    xT_d = nc.dram_tensor("xT", [128, KC, T], F32, kind="ExternalInput").ap()
    wqk_d = nc.dram_tensor("wqk", [128, KC, 2 * DG], F32, kind="ExternalInput").ap()
    wv_d = nc.dram_tensor("wv", [128, KC, DG], F32, kind="ExternalInput").ap()
    wout_d = nc.dram_tensor("wout", [128, 2, D], F32, kind="ExternalInput").ap()
    cos_d = nc.dram_tensor("cos128", [128, T], F32, kind="ExternalInput").ap()
    sin_d = nc.dram_tensor("sin128s", [128, T], F32, kind="ExternalInput").ap()
    tri_d = nc.dram_tensor("tri", [128, 128], F32, kind="ExternalInput").ap()
    out_d = nc.dram_tensor("out", [T, D], F32, kind="ExternalOutput").ap()

    with TileContext(nc) as tc:
        with (
            tc.tile_pool(name="const", bufs=1) as cpool,
            tc.tile_pool(name="big", bufs=1) as big,
            tc.tile_pool(name="work", bufs=2) as work,
            tc.tile_pool(name="expp", bufs=2) as expp,
            tc.tile_pool(name="outp", bufs=2) as outp,
            tc.tile_pool(name="ps_mm", bufs=2, space="PSUM") as ps_mm,
            tc.tile_pool(name="ps_sc", bufs=2, space="PSUM") as ps_sc,
            tc.tile_pool(name="ps_pv", bufs=1, space="PSUM") as ps_pv,
        ):
            cos_sb = cpool.tile([128, T], F32)
            sin_sb = cpool.tile([128, T], F32)
            tri_sb = cpool.tile([128, 128], F32R)
            xT_sb = big.tile([128, KC, T], F32R)
            wqk_sb = big.tile([128, KC, 2 * DG], F32R)
            wv_sb = big.tile([128, KC, DG], F32R)
            wout_sb = big.tile([128, 2, D], F32R)
            # inputs arrive pre-arranged in SBUF layout; first q/k matmuls
            # need xT chunk 0 + wqk, so those go first
            nc.sync.dma_start(out=xT_sb[:, :, 0:512], in_=xT_d[:, :, 0:512].bitcast(F32R))
            nc.sync.dma_start(out=wqk_sb[:], in_=wqk_d.bitcast(F32R))
            nc.sync.dma_start(out=wv_sb[:], in_=wv_d.bitcast(F32R))
            nc.sync.dma_start(out=cos_sb[:], in_=cos_d[:])
            nc.sync.dma_start(out=sin_sb[:], in_=sin_d[:])
            nc.sync.dma_start(out=tri_sb[:], in_=tri_d.bitcast(F32R))
            for n in range(1, NT):
                nc.sync.dma_start(
                    out=xT_sb[:, :, n * 512:(n + 1) * 512],
                    in_=xT_d[:, :, n * 512:(n + 1) * 512].bitcast(F32R),
                )
            nc.sync.dma_start(out=wout_sb[:], in_=wout_d.bitcast(F32R))

            # PE warm-up: dummy matmuls fill the DMA lead-in so HAM unthrottles
            # before the first real matmul
            warm_sb = cpool.tile([128, 256], F32R)
            nc.vector.memset(warm_sb[:].bitcast(F32), 0.0)
            for w in range(50):
                wp = ps_sc.tile([128, 256], F32, tag="sc")
                nc.tensor.matmul(
                    wp[:], lhsT=warm_sb[:, 0:128], rhs=warm_sb[:],
                    start=True, stop=True,
                )

            # qkT_sb m-index: 0,1 = q head-pairs (0,1),(2,3); 2,3 = k pairs
            qkT_sb = big.tile([128, 4, T], F32R)
            v_sb = big.tile([128, TT, HPG, HD + 1], F32R)
            nc.vector.memset(v_sb[:, :, :, HD].bitcast(F32), 1.0)
            outT_sb = big.tile([128, 2, T], F32R)

            def qkv_chunk(n):
                ns = slice(n * 512, (n + 1) * 512)
                # q,k projection (transposed) + RoPE
                for m in range(4):
                    ps = ps_mm.tile([128, 512], F32, tag="mm")
                    for k in range(KC):
                        nc.tensor.matmul(
                            ps[:],
                            lhsT=wqk_sb[:, k, m * 128:(m + 1) * 128],
                            rhs=xT_sb[:, k, ns],
                            start=(k == 0),
                            stop=(k == KC - 1),
                        )
                    # RoPE: rot = ps*cos + swap16(ps)*sin_signed
                    qk_raw = work.tile([128, 512], F32, tag="qk_raw", bufs=3)
                    swp = work.tile([128, 512], F32, tag="swp")
                    nc.vector.tensor_copy(qk_raw[:], ps[:])
                    nc.vector.stream_shuffle(swp[:], qk_raw[:], SWAP16)
                    nc.gpsimd.tensor_mul(qk_raw[:], qk_raw[:], cos_sb[:, ns])
                    nc.gpsimd.tensor_mul(swp[:], swp[:], sin_sb[:, ns])
                    nc.vector.tensor_add(qkT_sb[:, m, ns], qk_raw[:], swp[:])
                # v projection (natural layout)
                for j in range(4 * n, 4 * n + 4):
                    ps = ps_mm.tile([128, 256], F32, tag="mm")
                    for k in range(KC):
                        nc.tensor.matmul(
                            ps[:],
                            lhsT=xT_sb[:, k, j * 128:(j + 1) * 128],
                            rhs=wv_sb[:, k, :],
                            start=(k == 0),
                            stop=(k == KC - 1),
                        )
                    nc.vector.tensor_copy(
                        v_sb[:, j, :, 0:HD], ps[:].rearrange("p (h d) -> p h d", h=HPG)
                    )

            def attn_group(g):
                # heads 2hp (partitions 0:64) and 2hp+1 (64:128) packed per pass
                for hp in range(2):
                    qm, km = hp, 2 + hp
                    pv0 = ps_pv.tile([65, 512], F32, tag="pv0")
                    pv1 = ps_pv.tile([65, 512], F32, tag="pv1")
                    jmax = 4 * g + 3
                    for j in range(jmax + 1):
                        d = j - 4 * g
                        nstart = 128 * d if d > 0 else 0
                        ncols = 512 - nstart
                        ex = expp.tile([128, 1024], F32R, tag="ex")
                        # two heads' score matmuls packed into one PE pass
                        # (row groups 0-1 / 2-3), one wide exp over both
                        sc = ps_sc.tile([128, 1024], F32, tag="sc")
                        for half in range(2):
                            pb = 64 * half
                            nc.tensor.matmul(
                                sc[:, half * 512:half * 512 + ncols],
                                lhsT=qkT_sb[pb:pb + 64, km, j * 128:(j + 1) * 128],
                                rhs=qkT_sb[pb:pb + 64, qm, g * 512 + nstart:(g + 1) * 512],
                                start=True,
                                stop=True,
                            )
                        if ncols == 512:
                            nc.scalar.activation(
                                ex[:], sc[:],
                                mybir.ActivationFunctionType.Exp, scale=0.125,
                            )
                        else:
                            exv = ex[:].rearrange("p (u c) -> p u c", u=2)[:, :, 0:ncols]
                            scv = sc[:].rearrange("p (u c) -> p u c", u=2)[:, :, 0:ncols]
                            nc.scalar.activation(
                                exv, scv, mybir.ActivationFunctionType.Exp, scale=0.125,
                            )

                        if d >= 0:
                            nc.vector.tensor_mul(ex[:, 0:128], ex[:, 0:128], tri_sb[:])
                            nc.vector.tensor_mul(ex[:, 512:640], ex[:, 512:640], tri_sb[:])
                        for half, pv in ((0, pv0), (1, pv1)):
                            nc.tensor.matmul(
                                pv[:, nstart:512],
                                lhsT=v_sb[:, j, 2 * hp + half, :],
                                rhs=ex[:, half * 512:half * 512 + ncols],
                                start=(j == 0),
                                stop=(j == jmax),
                            )
                    for half, pv in ((0, pv0), (1, pv1)):
                        pb = 64 * half
                        den = work.tile([1, 512], F32, tag="den", bufs=1)
                        nc.vector.tensor_copy(den[:], pv[64:65, :])
                        rec = work.tile([1, 512], F32, tag="rec", bufs=1)
                        nc.vector.reciprocal_approx_fast(rec[:], den[:])
                        recb = work.tile([64, 512], F32, tag="recb", bufs=1)
                        nc.gpsimd.partition_broadcast(recb[:], rec[0:1, :], channels=64)
                        nc.vector.tensor_mul(
                            outT_sb[pb:pb + 64, hp, g * 512:(g + 1) * 512],
                            pv[0:64, :],
                            recb[:],
                        )

            def proj_group(g):
                for t in range(4 * g, 4 * g + 4):
                    for nh in range(2):
                        ps = ps_mm.tile([128, 512], F32, tag="mm")
                        for c in range(2):
                            nc.tensor.matmul(
                                ps[:],
                                lhsT=outT_sb[:, c, t * 128:(t + 1) * 128],
                                rhs=wout_sb[:, c, nh * 512:(nh + 1) * 512],
                                start=(c == 0),
                                stop=(c == 1),
                            )
                        ot = outp.tile([128, 512], F32, tag="ot")
                        if (t + nh) % 2 == 0:
                            nc.scalar.copy(out=ot[:], in_=ps[:])
                        else:
                            nc.vector.tensor_copy(ot[:], ps[:])
                        nc.sync.dma_start(
                            out=out_d[t * 128:(t + 1) * 128, nh * 512:(nh + 1) * 512],
                            in_=ot[:],
                        )

            # interleave: QKV chunk n+1 overlaps attention group n-1's ACT work
            qkv_chunk(0)
            qkv_chunk(1)
            attn_group(0)
            proj_group(0)
            qkv_chunk(2)
            attn_group(1)
            proj_group(1)
            qkv_chunk(3)
            attn_group(2)
            attn_group(3)
            proj_group(2)
            proj_group(3)

            if DEBUG_DUMPS:
                dq = nc.dram_tensor("dbg_qk", [128, 4, T], F32, kind="ExternalOutput").ap()
                dv = nc.dram_tensor("dbg_v", [128, TT, HPG, HD + 1], F32, kind="ExternalOutput").ap()
                do = nc.dram_tensor("dbg_outT", [128, 2, T], F32, kind="ExternalOutput").ap()
                nc.sync.dma_start(out=dq[:], in_=qkT_sb[:].bitcast(F32))
                nc.sync.dma_start(out=dv[:], in_=v_sb[:].bitcast(F32))
                nc.sync.dma_start(out=do[:], in_=outT_sb[:].bitcast(F32))

    nc.compile()
    return nc


def _qk_perm():
    """hd permutation for q/k columns: RoPE pair j -> (e,o) rows 16-interleaved
    so the swap stays within 32-partition quadrants (stream_shuffle-able)."""
    perm = np.empty(HD, dtype=np.int64)
    for p in range(HD):
        q32, i = divmod(p, 32)
        j = 16 * q32 + (i % 16)
        perm[p] = 2 * j + (1 if i >= 16 else 0)
    return perm


def _prepare_shards(x, w_qkv, w_out, freqs_cos, freqs_sin):
    perm = _qk_perm()
    cosT = np.ascontiguousarray(freqs_cos.T)  # [32, T]
    sinT = np.ascontiguousarray(freqs_sin.T)
    # row p of a 64-row head block: pair j = 16*(p//32 % 2) + p%16, sign -/+ for e/o
    cos128 = np.empty((128, T), dtype=np.float32)
    sin128s = np.empty((128, T), dtype=np.float32)
    for p in range(128):
        ph = p % 64
        q32, i = divmod(ph, 32)
        j = 16 * q32 + (i % 16)
        cos128[p] = cosT[j]
        sin128s[p] = sinT[j] * (-1.0 if i < 16 else 1.0)
    kk, qq = np.meshgrid(np.arange(128), np.arange(128), indexing="ij")
    tri = (kk <= qq).astype(np.float32)

    w3 = w_qkv.reshape(D, 3, H, HD)
    in_maps = []
    for core in range(8):
        b, g = divmod(core, G)
        heads = np.arange(g * HPG, (g + 1) * HPG)
        wq = w3[:, 0, heads][:, :, perm].reshape(D, DG)
        wk = w3[:, 1, heads][:, :, perm].reshape(D, DG)
        wqk = np.ascontiguousarray(np.concatenate([wq, wk], axis=1))
        wv = np.ascontiguousarray(w3[:, 2, heads].reshape(D, DG))
        wo = np.ascontiguousarray(w_out.reshape(H, HD, D)[heads].reshape(DG, D))
        def sb_layout(a, kc=KC):
            # [128*kc, F] -> [128, kc, F] with partition-major contiguity
            return np.ascontiguousarray(a.reshape(kc, 128, -1).transpose(1, 0, 2))
        in_maps.append({
            "xT": sb_layout(x[b].T),
            "wqk": sb_layout(wqk),
            "wv": sb_layout(wv),
            "wout": sb_layout(wo, kc=2),
            "cos128": cos128,
            "sin128s": sin128s,
            "tri": tri,
        })
    return in_maps


def _run(in_maps, **kw):
    if "nc" not in _CACHE:
        _CACHE["nc"] = _build()
    return run_bass_kernel_spmd(_CACHE["nc"], in_maps, core_ids=list(range(8)), **kw)


def kernel(x, w_qkv, w_out, freqs_cos, freqs_sin):
    x = np.asarray(x, dtype=np.float32)
    w_qkv = np.asarray(w_qkv, dtype=np.float32)
    w_out = np.asarray(w_out, dtype=np.float32)
    freqs_cos = np.asarray(freqs_cos, dtype=np.float32)
    freqs_sin = np.asarray(freqs_sin, dtype=np.float32)

    in_maps = _prepare_shards(x, w_qkv, w_out, freqs_cos, freqs_sin)
    res = _run(in_maps)
    out = np.zeros((B, T, D), dtype=np.float64)
    for core in range(8):
        out[core // G] += res.results[core]["out"].astype(np.float64)
    return out.astype(np.float32)



# revision 13
# speedup vs baseline: 1.2365x; 1.2365x over previous
"""Causal self-attention (B=2,T=2048,D=1024,H=16,HD=64) + RoPE on 8 TRN2 NeuronCores.

Sharding: core = b*4 + g  (b: batch, g: head-group of 4 heads).
Each core computes QKV projection for its 4 heads, causal attention, and a
partial out-projection (rank-256 contribution). Host sums the 4 partials per
batch (the "all-reduce after out_proj").

v2 schedule: all matmul operands in bf16 (same PE cycles as f32r but FWL
weight loads and half the DMA), QKV fully up-front as one dense PE stream,
then a software-pipelined attention loop (score j+1 issued before pv j so the
PE never waits on the exp), exp on ACT in bf16-out, RoPE muls on DVE, causal
masks as one broadcast DVE op, denominator broadcast via a K=2 selector
matmul, out-proj blocks slotted right after each group's normalization.
"""
import numpy as np

import concourse.bass as bass
import concourse.mybir as mybir
from concourse import bacc
from concourse.tile import TileContext
from concourse.bass_utils import run_bass_kernel_spmd

B, T, D, H = 2, 2048, 1024, 16
HD = D // H            # 64
G = 4                  # head groups (tensor-parallel factor)
HPG = H // G           # 4 heads per group
DG = HPG * HD          # 256 head-dims per group
KC = D // 128          # 8 contraction chunks for D
NT = T // 512          # 4 T-chunks of 512
TT = T // 128          # 16 T-tiles of 128
F32 = mybir.dt.float32
F32R = mybir.dt.float32r
BF16 = mybir.dt.bfloat16
SWAP16 = [(i + 16) % 32 for i in range(32)]  # e<->o halves within each 32-quadrant

_CACHE = {}
DEBUG_DUMPS = False


def _build():
    nc = bacc.Bacc("TRN2", target_bir_lowering=False, debug=False, num_devices=8)

    xT_d = nc.dram_tensor("xT", [128, KC, T], BF16, kind="ExternalInput").ap()
    wqk_d = nc.dram_tensor("wqk", [128, KC, 2 * DG], BF16, kind="ExternalInput").ap()
    wv_d = nc.dram_tensor("wv", [128, KC, DG], BF16, kind="ExternalInput").ap()
    wout_d = nc.dram_tensor("wout", [128, 2, D], BF16, kind="ExternalInput").ap()
    cos_d = nc.dram_tensor("cos128", [128, T], BF16, kind="ExternalInput").ap()
    sin_d = nc.dram_tensor("sin128s", [128, T], BF16, kind="ExternalInput").ap()
    tri_d = nc.dram_tensor("tri", [128, 128], BF16, kind="ExternalInput").ap()
    out_d = nc.dram_tensor("out", [T, D], BF16, kind="ExternalOutput").ap()

    with TileContext(nc) as tc:
        with (
            tc.tile_pool(name="const", bufs=1) as cpool,
            tc.tile_pool(name="big", bufs=1) as big,
            tc.tile_pool(name="work", bufs=2) as work,
            tc.tile_pool(name="expp", bufs=2) as expp,
            tc.tile_pool(name="outp", bufs=2) as outp,
            tc.tile_pool(name="ps_mm", bufs=2, space="PSUM") as ps_mm,
            tc.tile_pool(name="ps_sc", bufs=2, space="PSUM") as ps_sc,
            tc.tile_pool(name="ps_pv", bufs=1, space="PSUM") as ps_pv,
        ):
            cos_sb = cpool.tile([128, T], BF16)
            sin_sb = cpool.tile([128, T], BF16)
            tri_sb = cpool.tile([128, 128], BF16)
            # selector + den/rec rows live on partitions 0 and 64 (32-aligned)
            sel_sb = cpool.tile([128, 128], F32R)
            den2_sb = cpool.tile([65, 512], F32)
            rec2f_sb = cpool.tile([65, 512], F32)
            rec2_sb = cpool.tile([128, 512], F32R)
            xT_sb = big.tile([128, KC, T], BF16)
            wqk_sb = big.tile([128, KC, 2 * DG], BF16)
            wv_sb = big.tile([128, KC, DG], BF16)
            wout_sb = big.tile([128, 2, D], BF16)

            # DMA order = consumption order; first q/k m-tile k-loop streams
            # against the (wqk k, xT chunk-0 k) pairs. Spread issue across
            # queues so no single engine serializes ~30 descriptors.
            for k in range(4):
                nc.sync.dma_start(out=wqk_sb[:, k, :], in_=wqk_d[:, k, :])
                nc.sync.dma_start(out=xT_sb[:, k, 0:512], in_=xT_d[:, k, 0:512])
            for k in range(4, KC):
                nc.gpsimd.dma_start(out=wqk_sb[:, k, :], in_=wqk_d[:, k, :])
                nc.gpsimd.dma_start(out=xT_sb[:, k, 0:512], in_=xT_d[:, k, 0:512])
            nc.gpsimd.dma_start(out=cos_sb[:, 0:512], in_=cos_d[:, 0:512])
            nc.gpsimd.dma_start(out=sin_sb[:, 0:512], in_=sin_d[:, 0:512])
            nc.gpsimd.dma_start(out=wv_sb[:], in_=wv_d[:])
            nc.gpsimd.dma_start(out=tri_sb[:], in_=tri_d[:])
            nc.gpsimd.dma_start(
                out=xT_sb[:, :, 512:1024], in_=xT_d[:, :, 512:1024]
            )
            nc.sync.dma_start(out=xT_sb[:, :, 1024:1536], in_=xT_d[:, :, 1024:1536])
            nc.sync.dma_start(out=xT_sb[:, :, 1536:2048], in_=xT_d[:, :, 1536:2048])
            nc.sync.dma_start(out=cos_sb[:, 512:T], in_=cos_d[:, 512:T])
            nc.sync.dma_start(out=sin_sb[:, 512:T], in_=sin_d[:, 512:T])
            nc.sync.dma_start(out=wout_sb[:], in_=wout_d[:])

            # selector for the denominator broadcast: recb = sel.T @ rec2
            nc.vector.memset(sel_sb[:].bitcast(F32), 0.0)
            nc.vector.memset(sel_sb[0:1, 0:64].bitcast(F32), 1.0)
            nc.vector.memset(sel_sb[64:65, 64:128].bitcast(F32), 1.0)
            nc.vector.memset(rec2_sb[:].bitcast(F32), 0.0)
            nc.vector.memset(den2_sb[:], 1.0)

            # qkT_sb m-index: 0,1 = q head-pairs (0,1),(2,3); 2,3 = k pairs
            qkT_sb = big.tile([128, 4, T], BF16)
            v_sb = big.tile([128, TT, HPG, HD + 1], BF16)
            nc.vector.memset(v_sb[:, :, :, HD], 1.0)
            outT_sb = big.tile([128, 2, T], BF16)

            def qkv_chunk(n):
                ns = slice(n * 512, (n + 1) * 512)
                # q,k projection (transposed) + RoPE
                for m in range(4):
                    ps = ps_mm.tile([128, 512], F32, tag="mm")
                    for k in range(KC):
                        nc.tensor.matmul(
                            ps[:],
                            lhsT=wqk_sb[:, k, m * 128:(m + 1) * 128],
                            rhs=xT_sb[:, k, ns],
                            start=(k == 0),
                            stop=(k == KC - 1),
                        )
                    # RoPE: rot = raw*cos + swap16(raw)*sin_signed
                    qk_raw = work.tile([128, 512], BF16, tag="qk_raw", bufs=2)
                    swp = work.tile([128, 512], BF16, tag="swp")
                    nc.scalar.copy(out=qk_raw[:], in_=ps[:])
                    nc.vector.stream_shuffle(swp[:], qk_raw[:], SWAP16)
                    nc.vector.tensor_mul(qk_raw[:], qk_raw[:], cos_sb[:, ns])
                    nc.vector.tensor_mul(swp[:], swp[:], sin_sb[:, ns])
                    nc.vector.tensor_add(qkT_sb[:, m, ns], qk_raw[:], swp[:])
                # v projection (natural layout)
                for j in range(4 * n, 4 * n + 4):
                    ps = ps_mm.tile([128, 256], F32, tag="mm")
                    for k in range(KC):
                        nc.tensor.matmul(
                            ps[:],
                            lhsT=xT_sb[:, k, j * 128:(j + 1) * 128],
                            rhs=wv_sb[:, k, :],
                            start=(k == 0),
                            stop=(k == KC - 1),
                        )
                    nc.scalar.copy(
                        out=v_sb[:, j, :, 0:HD],
                        in_=ps[:].rearrange("p (h d) -> p h d", h=HPG),
                    )

            # ---- software-pipelined attention over passes (g, hp) ----
            passes = [(g, hp) for g in range(G) for hp in range(2)]
            seq = [(g, hp, j) for (g, hp) in passes for j in range(4 * g + 4)]
            pv_tiles = {}

            def emit_score(g, hp, j):
                d = j - 4 * g
                nstart = 128 * d if d > 0 else 0
                ncols = 512 - nstart
                qm, km = hp, 2 + hp
                sc = ps_sc.tile([128, 1024], F32, tag="sc")
                for half in range(2):
                    pb = 64 * half
                    nc.tensor.matmul(
                        sc[:, half * 512:half * 512 + ncols],
                        lhsT=qkT_sb[pb:pb + 64, km, j * 128:(j + 1) * 128],
                        rhs=qkT_sb[pb:pb + 64, qm, g * 512 + nstart:(g + 1) * 512],
                        start=True,
                        stop=True,
                    )
                ex = expp.tile([128, 1024], BF16, tag="ex")
                if ncols == 512:
                    nc.scalar.activation(
                        ex[:], sc[:], mybir.ActivationFunctionType.Exp, scale=0.125,
                    )
                else:
                    exv = ex[:].rearrange("p (u c) -> p u c", u=2)[:, :, 0:ncols]
                    scv = sc[:].rearrange("p (u c) -> p u c", u=2)[:, :, 0:ncols]
                    nc.scalar.activation(
                        exv, scv, mybir.ActivationFunctionType.Exp, scale=0.125,
                    )
                if d >= 0:
                    # causal mask on the leading 128 cols of both halves
                    exd = ex[:].rearrange("p (u c) -> p u c", u=2)[:, :, 0:128]
                    trib = tri_sb[:].unsqueeze(1).broadcast_to([128, 2, 128])
                    nc.vector.tensor_mul(exd, exd, trib)
                return ex

            def emit_pv(g, hp, j, ex):
                d = j - 4 * g
                nstart = 128 * d if d > 0 else 0
                ncols = 512 - nstart
                jmax = 4 * g + 3
                if j == 0:
                    pv_tiles[0] = ps_pv.tile([65, 512], F32, tag="pv0", name="pv0")
                    pv_tiles[1] = ps_pv.tile([65, 512], F32, tag="pv1", name="pv1")
                for half in range(2):
                    nc.tensor.matmul(
                        pv_tiles[half][:, nstart:512],
                        lhsT=v_sb[:, j, 2 * hp + half, :],
                        rhs=ex[:, half * 512:half * 512 + ncols],
                        start=(j == 0),
                        stop=(j == jmax),
                    )

            def emit_norm(g, hp):
                # den rows live at partition 64 of each pv tile (ones-col of v)
                nc.vector.tensor_copy(den2_sb[0:1, :], pv_tiles[0][64:65, :])
                nc.vector.tensor_copy(den2_sb[64:65, :], pv_tiles[1][64:65, :])
                nc.vector.reciprocal_approx_fast(rec2f_sb[:], den2_sb[:])
                nc.vector.tensor_copy(rec2_sb[0:65, :], rec2f_sb[:])
                recb = ps_mm.tile([128, 512], F32, tag="mm")
                nc.tensor.matmul(
                    recb[:],
                    lhsT=sel_sb[:],
                    rhs=rec2_sb[:],
                    start=True,
                    stop=True,
                )
                recb_sb = work.tile([128, 512], F32, tag="recb", bufs=2)
                nc.scalar.copy(out=recb_sb[:], in_=recb[:])
                for half in range(2):
                    pb = 64 * half
                    nc.vector.tensor_mul(
                        outT_sb[pb:pb + 64, hp, g * 512:(g + 1) * 512],
                        pv_tiles[half][0:64, :],
                        recb_sb[pb:pb + 64, :],
                    )

            def proj_group(g):
                for t in range(4 * g, 4 * g + 4):
                    for nh in range(2):
                        ps = ps_mm.tile([128, 512], F32, tag="mm")
                        for c in range(2):
                            nc.tensor.matmul(
                                ps[:],
                                lhsT=outT_sb[:, c, t * 128:(t + 1) * 128],
                                rhs=wout_sb[:, c, nh * 512:(nh + 1) * 512],
                                start=(c == 0),
                                stop=(c == 1),
                            )
                        ot = outp.tile([128, 512], BF16, tag="ot")
                        if (t + nh) % 2 == 0:
                            nc.scalar.copy(out=ot[:], in_=ps[:])
                        else:
                            nc.vector.tensor_copy(ot[:], ps[:])
                        nc.sync.dma_start(
                            out=out_d[t * 128:(t + 1) * 128, nh * 512:(nh + 1) * 512],
                            in_=ot[:],
                        )

            # dense QKV stream first; attention is ACT/PE-balanced after
            for n in range(NT):
                qkv_chunk(n)

            prev = None
            for i, (g, hp, j) in enumerate(seq):
                ex = emit_score(g, hp, j)
                if prev is not None:
                    pg, php, pj, pex = prev
                    emit_pv(pg, php, pj, pex)
                    if pj == 4 * pg + 3:
                        emit_norm(pg, php)
                        if php == 1:
                            proj_group(pg)
                prev = (g, hp, j, ex)
            pg, php, pj, pex = prev
            emit_pv(pg, php, pj, pex)
            emit_norm(pg, php)
            proj_group(G - 1)

            if DEBUG_DUMPS:
                dq = nc.dram_tensor("dbg_qk", [128, 4, T], BF16, kind="ExternalOutput").ap()
                dv = nc.dram_tensor("dbg_v", [128, TT, HPG, HD + 1], BF16, kind="ExternalOutput").ap()
                do = nc.dram_tensor("dbg_outT", [128, 2, T], BF16, kind="ExternalOutput").ap()
                dd = nc.dram_tensor("dbg_den2", [65, 512], F32, kind="ExternalOutput").ap()
                dr = nc.dram_tensor("dbg_rec2", [128, 512], F32, kind="ExternalOutput").ap()
                ds = nc.dram_tensor("dbg_sel", [128, 128], F32, kind="ExternalOutput").ap()
                nc.sync.dma_start(out=dq[:], in_=qkT_sb[:])
                nc.sync.dma_start(out=dv[:], in_=v_sb[:])
                nc.sync.dma_start(out=do[:], in_=outT_sb[:])
                nc.sync.dma_start(out=dd[:], in_=den2_sb[:])
                nc.sync.dma_start(out=dr[:], in_=rec2_sb[:].bitcast(F32))
                nc.sync.dma_start(out=ds[:], in_=sel_sb[:].bitcast(F32))

    nc.compile()
    return nc


def _qk_perm():
    """hd permutation for q/k columns: RoPE pair j -> (e,o) rows 16-interleaved
    so the swap stays within 32-partition quadrants (stream_shuffle-able)."""
    perm = np.empty(HD, dtype=np.int64)
    for p in range(HD):
        q32, i = divmod(p, 32)
        j = 16 * q32 + (i % 16)
        perm[p] = 2 * j + (1 if i >= 16 else 0)
    return perm


def _bf16(a):
    import ml_dtypes
    return np.asarray(a, dtype=np.float32).astype(ml_dtypes.bfloat16)


def _prepare_shards(x, w_qkv, w_out, freqs_cos, freqs_sin):
    perm = _qk_perm()
    cosT = np.ascontiguousarray(freqs_cos.T)  # [32, T]
    sinT = np.ascontiguousarray(freqs_sin.T)
    # row p of a 64-row head block: pair j = 16*(p//32 % 2) + p%16, sign -/+ for e/o
    cos128 = np.empty((128, T), dtype=np.float32)
    sin128s = np.empty((128, T), dtype=np.float32)
    for p in range(128):
        ph = p % 64
        q32, i = divmod(ph, 32)
        j = 16 * q32 + (i % 16)
        cos128[p] = cosT[j]
        sin128s[p] = sinT[j] * (-1.0 if i < 16 else 1.0)
    kk, qq = np.meshgrid(np.arange(128), np.arange(128), indexing="ij")
    tri = (kk <= qq).astype(np.float32)

    w3 = w_qkv.reshape(D, 3, H, HD)
    in_maps = []
    for core in range(8):
        b, g = divmod(core, G)
        heads = np.arange(g * HPG, (g + 1) * HPG)
        wq = w3[:, 0, heads][:, :, perm].reshape(D, DG)
        wk = w3[:, 1, heads][:, :, perm].reshape(D, DG)
        wqk = np.ascontiguousarray(np.concatenate([wq, wk], axis=1))
        wv = np.ascontiguousarray(w3[:, 2, heads].reshape(D, DG))
        wo = np.ascontiguousarray(w_out.reshape(H, HD, D)[heads].reshape(DG, D))
        def sb_layout(a, kc=KC):
            # [128*kc, F] -> [128, kc, F] with partition-major contiguity
            return np.ascontiguousarray(a.reshape(kc, 128, -1).transpose(1, 0, 2))
        in_maps.append({
            "xT": _bf16(sb_layout(x[b].T)),
            "wqk": _bf16(sb_layout(wqk)),
            "wv": _bf16(sb_layout(wv)),
            "wout": _bf16(sb_layout(wo, kc=2)),
            "cos128": _bf16(cos128),
            "sin128s": _bf16(sin128s),
            "tri": _bf16(tri),
        })
    return in_maps


def _run(in_maps, **kw):
    if "nc" not in _CACHE:
        _CACHE["nc"] = _build()
    return run_bass_kernel_spmd(_CACHE["nc"], in_maps, core_ids=list(range(8)), **kw)


def kernel(x, w_qkv, w_out, freqs_cos, freqs_sin):
    x = np.asarray(x, dtype=np.float32)
    w_qkv = np.asarray(w_qkv, dtype=np.float32)
    w_out = np.asarray(w_out, dtype=np.float32)
    freqs_cos = np.asarray(freqs_cos, dtype=np.float32)
    freqs_sin = np.asarray(freqs_sin, dtype=np.float32)

    in_maps = _prepare_shards(x, w_qkv, w_out, freqs_cos, freqs_sin)
    res = _run(in_maps)
    out = np.zeros((B, T, D), dtype=np.float64)
    for core in range(8):
        out[core // G] += np.asarray(res.results[core]["out"]).astype(np.float64)
    return out.astype(np.float32)
